# revision 12
# baseline (speedup 1.0000x reference)
"""Bass/Trainium2 kernel for nn_CrossAttention (two-direction cross attention).

Strategy (8 NeuronCores, SPMD, no collectives):
  - Direction split: cores 0-3 compute the c->p attention, cores 4-7 p->c.
    Within each direction the 4096 query rows are sharded 4 ways (1024
    rows/core); K/V inputs and weights are replicated per core
    (flash-attention row-block tiling per the sharding hint).
  - Algebraic folds (softmax drops per-query-row constants):
      scores:  S = (Q @ M + c) @ K_raw^T  with  M = Wq^T Wk,  c = bq Wk
               (M, c precomputed on the host from the weights; the bk bias
               only shifts each score row by a constant, so it is dropped)
      output:  out = (P @ V_raw) @ Wv^T  (Wv applied once to the 1024-row
               accumulated result in the epilogue)
  - Precision: the score matmul runs in fp8(e4m3) DoubleRow mode (2 MACs
    per PE cell per cycle); all other matmuls use bf16 operands with fp32
    PSUM accumulation.  fp8 anywhere in the P@V product would exceed the
    accuracy budget (measured by simulation), bf16 there is safe.
  - Per core: q2 = Q@M + c in bf16, quantized to fp8 by the activation
    that drains PSUM.  K^T streams in 256-key blocks as fp8; exp(S/sqrt(d))
    is written as bf16 into a full resident P^T tile [4096 keys x 1024
    queries].  V (bf16, resident) then contracts against P^T with a single
    PSUM accumulation chain over all 32 key subtiles per output tile - no
    vector-engine partial sums.  Softmax row sums come from a vector-engine
    reduction of P over key subtiles followed by a single 2-column matmul
    across key partitions.  Host applies out = PV / rowsum + bv.
"""

import numpy as np
import ml_dtypes

D = 1024          # d_in == d_out
N_FULL = 4096     # Nc == Np
N_CORES = 8
NQ = N_FULL // 4  # query rows per core (direction split 2 x 4)
KBLK = 256        # keys per streamed block
NKB = N_FULL // KBLK
DS = D // 128     # d subtiles (partition dim tiles)
KS = KBLK // 128  # key subtiles per block
NQT = NQ // 128   # query tiles
SCALE = 1.0 / float(np.sqrt(D))

_PROGRAM = None


# ---------------------------------------------------------------------------
# Environment patches: this container's walrus build rejects instructions
# carrying more than one semaphore wait ("Too many sync wait commands"), so
# after Tile scheduling we move excess waits onto single-wait NoOps inserted
# just before the instruction on the same engine. The agent image's antenv
# also lacks axon_hooks, which run_bass_kernel_spmd(trace=True) needs for
# NTFF profiling; recreate it.
# ---------------------------------------------------------------------------

def _install_patches():
    import concourse.tile as tile
    from concourse import mybir

    if getattr(tile.TileContext, "_multiwait_patched", False):
        return

    counter = [0]

    def split_multiwaits(nc):
        for fn in nc.m.functions:
            for bb in fn.blocks:
                new_list = []
                changed = False
                for inst in bb.instructions:
                    si = inst.sync_info
                    waits = list(si.on_wait) if si is not None else []
                    if len(waits) > 1:
                        changed = True
                        excess, keep = waits[:-1], waits[-1:]
                        for w in excess:
                            counter[0] += 1
                            new_list.append(
                                mybir.InstNoOp(
                                    name=f"I-waitsplit-{counter[0]}",
                                    engine=inst.engine,
                                    sync_info=mybir.SyncInfo(
                                        on_wait=[w], on_update=[]
                                    ),
                                )
                            )
                        si.on_wait[:] = keep
                    new_list.append(inst)
                if changed:
                    bb.instructions[:] = new_list

    orig_exit = tile.TileContext.__exit__

    def patched_exit(self, *args):
        r = orig_exit(self, *args)
        split_multiwaits(self.nc)
        return r

    tile.TileContext.__exit__ = patched_exit
    tile.TileContext._multiwait_patched = True


def _install_ntff_hook():
    import sys, types
    try:
        import antenv
    except ImportError:
        return
    if "antenv.axon_hooks" in sys.modules:
        return
    mod = types.ModuleType("antenv.axon_hooks")
    holder = [None]
    mod.set_axon_ntff_profile_hook = lambda h: holder.__setitem__(0, h)
    mod.get_axon_ntff_profile_hook = lambda: holder[0]
    sys.modules["antenv.axon_hooks"] = mod
    antenv.axon_hooks = mod
    try:
        from trn_agent_boot.trn_boot import _ntff_profile_via_ctypes
        mod.set_axon_ntff_profile_hook(
            _ntff_profile_via_ctypes("/opt/axon/libaxon_pjrt.so")
        )
    except Exception:
        pass


# ---------------------------------------------------------------------------
# Device program (identical for all 8 cores; data differs per core)
# ---------------------------------------------------------------------------

def _build_program():
    import concourse.bass as bass
    import concourse.tile as tile
    from concourse import mybir

    F32 = mybir.dt.float32
    BF16 = mybir.dt.bfloat16
    FP8 = mybir.dt.float8e4
    AF = mybir.ActivationFunctionType
    DR = mybir.MatmulPerfMode.DoubleRow

    nc = bass.Bass("TRN2", target_bir_lowering=False, debug=False)

    QT = nc.dram_tensor("QT", [D, NQ], BF16, kind="ExternalInput")
    KT = nc.dram_tensor("KT", [D, N_FULL], FP8, kind="ExternalInput")
    VT = nc.dram_tensor("VT", [N_FULL, D], BF16, kind="ExternalInput")
    MT = nc.dram_tensor("MT", [D, D], BF16, kind="ExternalInput")
    WVT = nc.dram_tensor("WVT", [D, D], BF16, kind="ExternalInput")
    CB = nc.dram_tensor("CB", [128, DS], F32, kind="ExternalInput")
    ONES = nc.dram_tensor("ONES", [128, 128], BF16, kind="ExternalInput")
    OUT = nc.dram_tensor("OUT", [NQ, D], F32, kind="ExternalOutput")
    RS = nc.dram_tensor("RS", [2, NQ], F32, kind="ExternalOutput")

    qt_dram = QT.ap().rearrange("(s p) n -> p s n", p=128)
    kt_dram = KT.ap().rearrange("(s p) n -> p s n", p=128)
    v_dram = VT.ap().rearrange("(s p) d -> p s d", p=128)
    mt_dram = MT.ap().rearrange("(s p) d -> p s d", p=128)

    with tile.TileContext(nc) as tc:
        with (
            tc.tile_pool(name="persist", bufs=1) as persist,
            tc.tile_pool(name="kin", bufs=3) as kin,
            tc.tile_pool(name="ob", bufs=2) as ob,
            tc.tile_pool(name="ps_s", bufs=4, space="PSUM") as ps_s,
            tc.tile_pool(name="ps_pv", bufs=4, space="PSUM") as ps_pv,
        ):
            cb = persist.tile([128, DS], F32)
            nc.sync.dma_start(cb[:], CB.ap())
            ones = persist.tile([128, 128], BF16)
            nc.sync.dma_start(ones[:], ONES.ap())

            # M and Q^T land as fine-grained DMAs ordered so the first q2
            # chain (chunk 0, m 0) only waits on ~1MB.
            mt = persist.tile([128, DS, D], BF16, tag="mt")
            qin = persist.tile([128, DS, NQ], BF16)
            for j in range(DS):
                nc.sync.dma_start(qin[:, j, :], qt_dram[:, j, :])
                nc.sync.dma_start(mt[:, j, 0:512], mt_dram[:, j, 0:512])
            for j in range(DS):
                nc.sync.dma_start(mt[:, j, 512:D], mt_dram[:, j, 512:D])

            q2t = persist.tile([128, DS, NQ], FP8)
            pt = persist.tile([128, 2 * NKB, NQ], BF16)
            vsb = persist.tile([128, 2 * NKB, D], BF16)
            wvt = persist.tile([128, DS, D], BF16)
            rs_sb = persist.tile([2, NQ], F32)
            acc = persist.tile([128, NQ], F32)

            # ---- q2[d, nq] = M^T @ Q^T + c, written straight to fp8.
            # FD=256 chunks so the first chain starts on minimal DMA.
            for ch in range(NQ // 256):
                for m in range(DS):
                    psum = ps_s.tile([128, 512], F32, tag="s")
                    for j in range(DS):
                        nc.tensor.matmul(
                            psum[:, 0:256],
                            mt[:, j, m * 128:(m + 1) * 128],
                            qin[:, j, ch * 256:(ch + 1) * 256],
                            start=(j == 0),
                            stop=(j == DS - 1),
                        )
                    nc.scalar.activation(
                        q2t[:, m, ch * 256:(ch + 1) * 256], psum[:, 0:256],
                        AF.Identity, bias=cb[:, m:m + 1],
                    )

            # ---- scores: S^T[key, query] = K q2^T in fp8 DoubleRow
            # (contraction pairs of d-subtiles), exp -> resident P^T (bf16).
            # V and Wv^T stream in behind the K blocks for the later phases.
            for kb in range(NKB):
                ktin = kin.tile([128, DS, KBLK], FP8, tag="kin")
                nc.sync.dma_start(
                    ktin[:], kt_dram[:, :, kb * KBLK:(kb + 1) * KBLK]
                )
                for mk in range(KS):
                    for qb in range(NQ // 512):
                        psum = ps_s.tile([128, 512], F32, tag="s")
                        for jp in range(DS // 2):
                            nc.tensor.matmul(
                                psum[:],
                                ktin[:, 2 * jp:2 * jp + 2,
                                     mk * 128:(mk + 1) * 128],
                                q2t[:, 2 * jp:2 * jp + 2,
                                    qb * 512:(qb + 1) * 512],
                                start=(jp == 0),
                                stop=(jp == DS // 2 - 1),
                                perf_mode=DR,
                            )
                        nc.scalar.activation(
                            pt[:, kb * KS + mk, qb * 512:(qb + 1) * 512],
                            psum[:], AF.Exp, bias=ones[:, 8:9], scale=SCALE,
                        )
                # fold this block's P into the per-key partial row sums on
                # the (otherwise idle) vector engine
                for mk in range(KS):
                    if kb == 0 and mk == 0:
                        nc.vector.tensor_copy(acc[:], pt[:, 0, :])
                    else:
                        nc.vector.tensor_add(
                            acc[:], acc[:], pt[:, kb * KS + mk, :]
                        )
                nc.sync.dma_start(
                    vsb[:, kb * KS:(kb + 1) * KS, :],
                    v_dram[:, kb * KS:(kb + 1) * KS, :],
                )
                if kb == 4:
                    nc.sync.dma_start(
                        wvt[:], WVT.ap().rearrange("(s p) d -> p s d", p=128)
                    )

            # ---- PV: (P@V)^T[d, nq] = V^T P^T, one PSUM accumulation chain
            # over all 32 key subtiles per output tile.  Results are
            # rounded to bf16 for the epilogue; pvt_r reuses mt's SBUF.
            pvt_r = persist.tile([128, DS, NQ], BF16, tag="mt")
            for md in range(DS):
                for qb in range(NQ // 512):
                    psum = ps_pv.tile([128, 512], F32, tag="pv")
                    for ks in range(2 * NKB):
                        nc.tensor.matmul(
                            psum[:],
                            vsb[:, ks, md * 128:(md + 1) * 128],
                            pt[:, ks, qb * 512:(qb + 1) * 512],
                            start=(ks == 0),
                            stop=(ks == 2 * NKB - 1),
                        )
                    nc.scalar.activation(
                        pvt_r[:, md, qb * 512:(qb + 1) * 512],
                        psum[:], AF.Copy,
                    )
            # softmax row sums: the DVE already reduced the 32 key subtiles
            # into acc; one 2-wide matmul sums acc across key partitions.
            acc_b = persist.tile([128, NQ], BF16)
            nc.scalar.activation(acc_b[:], acc[:], AF.Copy)
            for qb in range(NQ // 512):
                psum = ps_pv.tile([128, 512], F32, tag="pv")
                nc.tensor.matmul(
                    psum[0:2, :],
                    ones[:, 0:2],
                    acc_b[:, qb * 512:(qb + 1) * 512],
                    start=True,
                    stop=True,
                )
                nc.scalar.activation(
                    rs_sb[0:2, qb * 512:(qb + 1) * 512],
                    psum[0:2, :], AF.Copy,
                )
            nc.sync.dma_start(RS.ap(), rs_sb[:])

            # ---- epilogue: OUT[nq, d_out] = (P@V) @ Wv^T
            out_dram = OUT.ap().rearrange("(m p) d -> p m d", p=128)
            for mq in range(NQT):
                for db in range(D // 512):
                    psum = ps_s.tile([128, 512], F32, tag="s")
                    for j in range(DS):
                        nc.tensor.matmul(
                            psum[:],
                            pvt_r[:, j, mq * 128:(mq + 1) * 128],
                            wvt[:, j, db * 512:(db + 1) * 512],
                            start=(j == 0),
                            stop=(j == DS - 1),
                        )
                    out_sb = ob.tile([128, 512], F32, tag="ob")
                    nc.scalar.activation(out_sb[:], psum[:], AF.Copy)
                    nc.sync.dma_start(
                        out_dram[:, mq, db * 512:(db + 1) * 512], out_sb[:]
                    )

    return nc


def _get_program():
    global _PROGRAM
    if _PROGRAM is None:
        _install_patches()
        _install_ntff_hook()
        _PROGRAM = _build_program()
    return _PROGRAM


# ---------------------------------------------------------------------------
# Host driver
# ---------------------------------------------------------------------------

def _bf(a):
    return np.ascontiguousarray(np.asarray(a, dtype=np.float32)).astype(
        ml_dtypes.bfloat16
    )


def _bft(a):
    return np.ascontiguousarray(
        np.asarray(a, dtype=np.float32).T
    ).astype(ml_dtypes.bfloat16)


def _fp8t(a):
    at = np.ascontiguousarray(np.asarray(a, dtype=np.float32).T)
    return np.clip(at, -240.0, 240.0).astype(ml_dtypes.float8_e4m3)


def _bias_tile(b):
    return np.ascontiguousarray(
        np.asarray(b, dtype=np.float32).reshape(DS, 128).T
    )


def _run(inputs, trace=False):
    from concourse.bass_utils import run_bass_kernel_spmd

    nc = _get_program()

    f32 = lambda k: np.asarray(inputs[k], dtype=np.float32)
    Qc, Qp = f32("Qc"), f32("Qp")
    ones = np.zeros((128, 128), ml_dtypes.bfloat16)
    ones[:, 0:2] = 1.0

    def common(Wq, Wk, Wv, bq, K, V):
        Wq, Wk, bq = map(np.asarray, (Wq, Wk, bq))
        M = Wq.astype(np.float32).T @ Wk.astype(np.float32)
        c = bq.astype(np.float32) @ Wk.astype(np.float32)
        return {
            "KT": _fp8t(K), "VT": _bf(V),
            "MT": _bf(M), "WVT": _bft(Wv),
            "CB": _bias_tile(c), "ONES": ones,
        }

    cp_common = common(inputs["Wq_c"], inputs["Wk_p"], inputs["Wv_p"],
                       inputs["bq_c"], inputs["Kp"], inputs["Vp"])
    pc_common = common(inputs["Wq_p"], inputs["Wk_c"], inputs["Wv_c"],
                       inputs["bq_p"], inputs["Kc"], inputs["Vc"])

    in_maps = []
    for i in range(4):
        in_maps.append({"QT": _bft(Qc[i * NQ:(i + 1) * NQ, :]), **cp_common})
    for i in range(4):
        in_maps.append({"QT": _bft(Qp[i * NQ:(i + 1) * NQ, :]), **pc_common})

    res = run_bass_kernel_spmd(
        nc, in_maps, core_ids=list(range(N_CORES)), trace=trace
    )

    def assemble(core_lo, bv):
        outs, rss = [], []
        for i in range(core_lo, core_lo + 4):
            r = res.results[i]
            outs.append(np.asarray(r["OUT"], dtype=np.float32))
            rs = np.asarray(r["RS"], dtype=np.float32)
            rss.append(rs[0])
        pv = np.concatenate(outs, axis=0)
        rs = np.concatenate(rss, axis=0)
        return pv / rs[:, None] + np.asarray(bv, dtype=np.float32)[None, :]

    comp_fused = assemble(0, inputs["bv_p"])
    prot_fused = assemble(4, inputs["bv_c"])
    return (comp_fused, prot_fused), res.exec_time_ns


def kernel(**inputs):
    (comp_fused, prot_fused), _ = _run(inputs, trace=False)
    return comp_fused, prot_fused


def kernel_traced(**inputs):
    """Like kernel() but also returns the profiled hardware execution time
    (ns, slowest traced core) for benchmarking."""
    return _run(inputs, trace=True)


# revision 13
# speedup vs baseline: 1.1810x; 1.1810x over previous
"""Bass/Trainium2 kernel for nn_CrossAttention (two-direction cross attention).

Strategy (8 NeuronCores, SPMD, no collectives):
  - Direction split: cores 0-3 compute the c->p attention, cores 4-7 p->c.
    Within each direction the 4096 query rows are sharded 4 ways (1024
    rows/core); K/V inputs and weights are replicated per core
    (flash-attention row-block tiling per the sharding hint).
  - Algebraic folds (softmax drops per-query-row constants):
      scores:  S = (Q @ M + c) @ K_raw^T  with  M = Wq^T Wk,  c = bq Wk
               (M, c precomputed on the host from the weights; the bk bias
               only shifts each score row by a constant, so it is dropped)
      output:  out = (P @ V_raw) @ Wv^T  (Wv applied once to the 1024-row
               accumulated result in the epilogue)
  - Precision: the score matmul runs in fp8(e4m3) DoubleRow mode (2 MACs
    per PE cell per cycle); all other matmuls use bf16 operands with fp32
    PSUM accumulation.  fp8 anywhere in the P@V product would exceed the
    accuracy budget (measured by simulation), bf16 there is safe.
  - Per core: q2 = Q@M + c in bf16, quantized to fp8 by the activation
    that drains PSUM.  K^T streams in 256-key blocks as fp8; exp(S/sqrt(d))
    is written as bf16 into a full resident P^T tile [4096 keys x 1024
    queries].  V (bf16, resident) then contracts against P^T with a single
    PSUM accumulation chain over all 32 key subtiles per output tile - no
    vector-engine partial sums.  Softmax row sums come from a vector-engine
    reduction of P over key subtiles followed by a single 2-column matmul
    across key partitions.  Host applies out = PV / rowsum + bv.
"""

import numpy as np
import ml_dtypes

D = 1024          # d_in == d_out
N_FULL = 4096     # Nc == Np
N_CORES = 8
NQ = N_FULL // 4  # query rows per core (direction split 2 x 4)
KBLK = 256        # keys per streamed block
NKB = N_FULL // KBLK
DS = D // 128     # d subtiles (partition dim tiles)
KS = KBLK // 128  # key subtiles per block
NQT = NQ // 128   # query tiles
SCALE = 1.0 / float(np.sqrt(D))

_PROGRAM = None


# ---------------------------------------------------------------------------
# Environment patches: this container's walrus build rejects instructions
# carrying more than one semaphore wait ("Too many sync wait commands"), so
# after Tile scheduling we move excess waits onto single-wait NoOps inserted
# just before the instruction on the same engine. The agent image's antenv
# also lacks axon_hooks, which run_bass_kernel_spmd(trace=True) needs for
# NTFF profiling; recreate it.
# ---------------------------------------------------------------------------

def _install_patches():
    import concourse.tile as tile
    from concourse import mybir

    if getattr(tile.TileContext, "_multiwait_patched", False):
        return

    counter = [0]

    def split_multiwaits(nc):
        for fn in nc.m.functions:
            for bb in fn.blocks:
                new_list = []
                changed = False
                for inst in bb.instructions:
                    si = inst.sync_info
                    waits = list(si.on_wait) if si is not None else []
                    if len(waits) > 1:
                        changed = True
                        excess, keep = waits[:-1], waits[-1:]
                        for w in excess:
                            counter[0] += 1
                            new_list.append(
                                mybir.InstNoOp(
                                    name=f"I-waitsplit-{counter[0]}",
                                    engine=inst.engine,
                                    sync_info=mybir.SyncInfo(
                                        on_wait=[w], on_update=[]
                                    ),
                                )
                            )
                        si.on_wait[:] = keep
                    new_list.append(inst)
                if changed:
                    bb.instructions[:] = new_list

    orig_exit = tile.TileContext.__exit__

    def patched_exit(self, *args):
        r = orig_exit(self, *args)
        split_multiwaits(self.nc)
        return r

    tile.TileContext.__exit__ = patched_exit
    tile.TileContext._multiwait_patched = True


def _install_ntff_hook():
    import sys, types
    try:
        import antenv
    except ImportError:
        return
    if "antenv.axon_hooks" in sys.modules:
        return
    mod = types.ModuleType("antenv.axon_hooks")
    holder = [None]
    mod.set_axon_ntff_profile_hook = lambda h: holder.__setitem__(0, h)
    mod.get_axon_ntff_profile_hook = lambda: holder[0]
    sys.modules["antenv.axon_hooks"] = mod
    antenv.axon_hooks = mod
    try:
        from trn_agent_boot.trn_boot import _ntff_profile_via_ctypes
        mod.set_axon_ntff_profile_hook(
            _ntff_profile_via_ctypes("/opt/axon/libaxon_pjrt.so")
        )
    except Exception:
        pass


# ---------------------------------------------------------------------------
# Device program (identical for all 8 cores; data differs per core)
# ---------------------------------------------------------------------------

def _build_program():
    import concourse.bass as bass
    import concourse.tile as tile
    from concourse import mybir

    F32 = mybir.dt.float32
    BF16 = mybir.dt.bfloat16
    FP8 = mybir.dt.float8e4
    AF = mybir.ActivationFunctionType
    DR = mybir.MatmulPerfMode.DoubleRow

    nc = bass.Bass("TRN2", target_bir_lowering=False, debug=False)

    QT = nc.dram_tensor("QT", [D, NQ], BF16, kind="ExternalInput")
    KT = nc.dram_tensor("KT", [D, N_FULL], FP8, kind="ExternalInput")
    VT = nc.dram_tensor("VT", [N_FULL, D], BF16, kind="ExternalInput")
    MT = nc.dram_tensor("MT", [D, D], BF16, kind="ExternalInput")
    WVT = nc.dram_tensor("WVT", [D, D], BF16, kind="ExternalInput")
    CB = nc.dram_tensor("CB", [128, DS], F32, kind="ExternalInput")
    ONES = nc.dram_tensor("ONES", [128, 128], BF16, kind="ExternalInput")
    OUT = nc.dram_tensor("OUT", [NQ, D], F32, kind="ExternalOutput")
    RS = nc.dram_tensor("RS", [2, NQ], F32, kind="ExternalOutput")

    qt_dram = QT.ap().rearrange("(s p) n -> p s n", p=128)
    kt_dram = KT.ap().rearrange("(s p) n -> p s n", p=128)
    v_dram = VT.ap().rearrange("(s p) d -> p s d", p=128)
    mt_dram = MT.ap().rearrange("(s p) d -> p s d", p=128)

    with tile.TileContext(nc) as tc:
        with (
            tc.tile_pool(name="persist", bufs=1) as persist,
            tc.tile_pool(name="kin", bufs=3) as kin,
            tc.tile_pool(name="ob", bufs=2) as ob,
            tc.tile_pool(name="ps_s", bufs=4, space="PSUM") as ps_s,
            tc.tile_pool(name="ps_pv", bufs=4, space="PSUM") as ps_pv,
        ):
            cb = persist.tile([128, DS], F32)
            nc.sync.dma_start(cb[:], CB.ap())
            ones = persist.tile([128, 128], BF16)
            nc.sync.dma_start(ones[:], ONES.ap())

            # M and Q^T land as fine-grained DMAs ordered so the first q2
            # chain (chunk 0, m 0) only waits on ~1MB.
            mt = persist.tile([128, DS, D], BF16, tag="mt")
            qin = persist.tile([128, DS, NQ], BF16)
            for j in range(DS):
                nc.sync.dma_start(qin[:, j, :], qt_dram[:, j, :])
                nc.sync.dma_start(mt[:, j, 0:512], mt_dram[:, j, 0:512])
            for j in range(DS):
                nc.sync.dma_start(mt[:, j, 512:D], mt_dram[:, j, 512:D])

            q2t = persist.tile([128, DS, NQ], FP8)
            pt = persist.tile([128, 2 * NKB, NQ], BF16)
            vsb = persist.tile([128, 2 * NKB, D], BF16)
            wvt = persist.tile([128, DS, D], BF16)
            rs_sb = persist.tile([2, NQ], F32)
            acc = persist.tile([128, NQ], F32)

            # ---- q2[d, nq] = M^T @ Q^T + c, written straight to fp8.
            # FD=256 chunks so the first chain starts on minimal DMA.
            for ch in range(NQ // 256):
                for m in range(DS):
                    psum = ps_s.tile([128, 512], F32, tag="s")
                    for j in range(DS):
                        nc.tensor.matmul(
                            psum[:, 0:256],
                            mt[:, j, m * 128:(m + 1) * 128],
                            qin[:, j, ch * 256:(ch + 1) * 256],
                            start=(j == 0),
                            stop=(j == DS - 1),
                        )
                    nc.scalar.activation(
                        q2t[:, m, ch * 256:(ch + 1) * 256], psum[:, 0:256],
                        AF.Identity, bias=cb[:, m:m + 1],
                    )

            # ---- scores: S^T[key, query] = K q2^T in fp8 DoubleRow
            # (contraction pairs of d-subtiles), exp -> resident P^T (bf16).
            # V and Wv^T stream in behind the K blocks for the later phases.
            for kb in range(NKB):
                ktin = kin.tile([128, DS, KBLK], FP8, tag="kin")
                nc.sync.dma_start(
                    ktin[:], kt_dram[:, :, kb * KBLK:(kb + 1) * KBLK]
                )
                for mk in range(KS):
                    for qb in range(NQ // 512):
                        psum = ps_s.tile([128, 512], F32, tag="s")
                        for jp in range(DS // 2):
                            nc.tensor.matmul(
                                psum[:],
                                ktin[:, 2 * jp:2 * jp + 2,
                                     mk * 128:(mk + 1) * 128],
                                q2t[:, 2 * jp:2 * jp + 2,
                                    qb * 512:(qb + 1) * 512],
                                start=(jp == 0),
                                stop=(jp == DS // 2 - 1),
                                perf_mode=DR,
                            )
                        nc.scalar.activation(
                            pt[:, kb * KS + mk, qb * 512:(qb + 1) * 512],
                            psum[:], AF.Exp, scale=SCALE,
                        )
                # fold this block's P into the per-key partial row sums on
                # the (otherwise idle) vector engine
                for mk in range(KS):
                    if kb == 0 and mk == 0:
                        nc.vector.tensor_copy(acc[:], pt[:, 0, :])
                    else:
                        nc.vector.tensor_add(
                            acc[:], acc[:], pt[:, kb * KS + mk, :]
                        )
                nc.sync.dma_start(
                    vsb[:, kb * KS:(kb + 1) * KS, :],
                    v_dram[:, kb * KS:(kb + 1) * KS, :],
                )
                if kb == 4:
                    nc.sync.dma_start(
                        wvt[:], WVT.ap().rearrange("(s p) d -> p s d", p=128)
                    )

            # ---- PV: (P@V)^T[d, nq] = V^T P^T, one PSUM accumulation chain
            # over all 32 key subtiles per output tile.  Results are
            # rounded to bf16 for the epilogue; pvt_r reuses mt's SBUF.
            pvt_r = persist.tile([128, DS, NQ], BF16, tag="mt")
            for md in range(DS):
                for qb in range(NQ // 512):
                    psum = ps_pv.tile([128, 512], F32, tag="pv")
                    for ks in range(2 * NKB):
                        nc.tensor.matmul(
                            psum[:],
                            vsb[:, ks, md * 128:(md + 1) * 128],
                            pt[:, ks, qb * 512:(qb + 1) * 512],
                            start=(ks == 0),
                            stop=(ks == 2 * NKB - 1),
                        )
                    nc.scalar.activation(
                        pvt_r[:, md, qb * 512:(qb + 1) * 512],
                        psum[:], AF.Identity,
                    )
            # softmax row sums: the DVE already reduced the 32 key subtiles
            # into acc; one 2-wide matmul sums acc across key partitions.
            acc_b = persist.tile([128, NQ], BF16)
            nc.scalar.activation(acc_b[:], acc[:], AF.Identity)
            for qb in range(NQ // 512):
                psum = ps_pv.tile([128, 512], F32, tag="pv")
                nc.tensor.matmul(
                    psum[0:2, :],
                    ones[:, 0:2],
                    acc_b[:, qb * 512:(qb + 1) * 512],
                    start=True,
                    stop=True,
                )
                nc.scalar.activation(
                    rs_sb[0:2, qb * 512:(qb + 1) * 512],
                    psum[0:2, :], AF.Identity,
                )
            nc.sync.dma_start(RS.ap(), rs_sb[:])

            # ---- epilogue: OUT[nq, d_out] = (P@V) @ Wv^T
            out_dram = OUT.ap().rearrange("(m p) d -> p m d", p=128)
            for mq in range(NQT):
                for db in range(D // 512):
                    psum = ps_s.tile([128, 512], F32, tag="s")
                    for j in range(DS):
                        nc.tensor.matmul(
                            psum[:],
                            pvt_r[:, j, mq * 128:(mq + 1) * 128],
                            wvt[:, j, db * 512:(db + 1) * 512],
                            start=(j == 0),
                            stop=(j == DS - 1),
                        )
                    out_sb = ob.tile([128, 512], F32, tag="ob")
                    nc.scalar.activation(out_sb[:], psum[:], AF.Identity)
                    nc.sync.dma_start(
                        out_dram[:, mq, db * 512:(db + 1) * 512], out_sb[:]
                    )

    return nc


def _get_program():
    global _PROGRAM
    if _PROGRAM is None:
        _install_patches()
        _install_ntff_hook()
        _PROGRAM = _build_program()
    return _PROGRAM


# ---------------------------------------------------------------------------
# Host driver
# ---------------------------------------------------------------------------

def _bf(a):
    return np.ascontiguousarray(np.asarray(a, dtype=np.float32)).astype(
        ml_dtypes.bfloat16
    )


def _bft(a):
    return np.ascontiguousarray(
        np.asarray(a, dtype=np.float32).T
    ).astype(ml_dtypes.bfloat16)


def _fp8t(a):
    at = np.ascontiguousarray(np.asarray(a, dtype=np.float32).T)
    return np.clip(at, -240.0, 240.0).astype(ml_dtypes.float8_e4m3)


def _bias_tile(b):
    return np.ascontiguousarray(
        np.asarray(b, dtype=np.float32).reshape(DS, 128).T
    )


def _run(inputs, trace=False):
    from concourse.bass_utils import run_bass_kernel_spmd

    nc = _get_program()

    f32 = lambda k: np.asarray(inputs[k], dtype=np.float32)
    Qc, Qp = f32("Qc"), f32("Qp")
    ones = np.zeros((128, 128), ml_dtypes.bfloat16)
    ones[:, 0:2] = 1.0

    def common(Wq, Wk, Wv, bq, K, V):
        Wq, Wk, bq = map(np.asarray, (Wq, Wk, bq))
        M = Wq.astype(np.float32).T @ Wk.astype(np.float32)
        c = bq.astype(np.float32) @ Wk.astype(np.float32)
        return {
            "KT": _fp8t(K), "VT": _bf(V),
            "MT": _bf(M), "WVT": _bft(Wv),
            "CB": _bias_tile(c), "ONES": ones,
        }

    cp_common = common(inputs["Wq_c"], inputs["Wk_p"], inputs["Wv_p"],
                       inputs["bq_c"], inputs["Kp"], inputs["Vp"])
    pc_common = common(inputs["Wq_p"], inputs["Wk_c"], inputs["Wv_c"],
                       inputs["bq_p"], inputs["Kc"], inputs["Vc"])

    in_maps = []
    for i in range(4):
        in_maps.append({"QT": _bft(Qc[i * NQ:(i + 1) * NQ, :]), **cp_common})
    for i in range(4):
        in_maps.append({"QT": _bft(Qp[i * NQ:(i + 1) * NQ, :]), **pc_common})

    res = run_bass_kernel_spmd(
        nc, in_maps, core_ids=list(range(N_CORES)), trace=trace
    )

    def assemble(core_lo, bv):
        outs, rss = [], []
        for i in range(core_lo, core_lo + 4):
            r = res.results[i]
            outs.append(np.asarray(r["OUT"], dtype=np.float32))
            rs = np.asarray(r["RS"], dtype=np.float32)
            rss.append(rs[0])
        pv = np.concatenate(outs, axis=0)
        rs = np.concatenate(rss, axis=0)
        return pv / rs[:, None] + np.asarray(bv, dtype=np.float32)[None, :]

    comp_fused = assemble(0, inputs["bv_p"])
    prot_fused = assemble(4, inputs["bv_c"])
    return (comp_fused, prot_fused), res.exec_time_ns


def kernel(**inputs):
    (comp_fused, prot_fused), _ = _run(inputs, trace=False)
    return comp_fused, prot_fused


def kernel_traced(**inputs):
    """Like kernel() but also returns the profiled hardware execution time
    (ns, slowest traced core) for benchmarking."""
    return _run(inputs, trace=True)
